# revision 7
# baseline (speedup 1.0000x reference)
"""Trainium2 Bass kernel for CompositionalFC (moe_routing) — fp8 DoubleRow.

Reference computation:
    z[n,b,o] = x[b,i] @ weight[n,i,o] + bias[n,o]
    out[b,o] = relu( sum_n comp_weight[b,n] * z[n,b,o] )

Strategy: data-parallel over batch across 8 NeuronCores (512 rows each,
weight/bias replicated). Matmuls run in fp8(e4m3) DoubleRow mode (2 fp8
MACs/PE/cycle, contracting 256 per instruction). Accuracy is preserved by
mean-centering the weights before quantization:
    W[n] = m[n] + V[n],   m[n,o] = mean_i W[n,i,o]
    x @ W[n] = S * m[n] + x @ V[n],   S[b] = sum_i x[b,i]
V spans [-0.5, 0.5] so e4m3 quantizes it ~2x better than raw U[0,1) W; the
rank-1 mean term plus the bias combine is restored exactly through a K=32
bf16 correction matmul:  corr = [c*S, c] @ [m; bias].

The correction is folded into expert 0's PSUM accumulation group by
pre-dividing it by c[:,0] on the host:  acc = (corr/c0 + z_0) * c0.
This removes the separate seed+copy phase and gives the PE warm-up work
that only needs the (tiny) correction tensors.

DMA triggers cost ~0.6us of serial sequencer time + ~1us of descriptor
generation each, so transfers are batched: one DMA per expert weight
(1 MiB), one for x, one merged correction tensor, per-batch-tile output
stores.
"""

import sys

for _p in ("/opt/trn_rl_repo",):
    if _p not in sys.path:
        sys.path.insert(0, _p)

from contextlib import ExitStack

import ml_dtypes
import numpy as np

import concourse.bass as bass
import concourse.mybir as mybir
import concourse.tile as tile
from concourse import bacc
from concourse.bass_utils import run_bass_kernel_spmd
from concourse.tile_rust import add_dep_helper

N_CORES = 8
BATCH, IN_DIM, OUT_DIM, N_EXP = 4096, 1024, 1024, 16
BS = BATCH // N_CORES          # 512 batch rows per core
P = 128                        # partitions
BT = BS // P                   # 4 batch tiles per core
KP = IN_DIM // 256             # 4 DoubleRow contraction chunks (256 each)
FD = 512                       # matmul free dim / PSUM bank width (fp32)
NO = OUT_DIM // FD             # 2 output column tiles
KC = 128                       # correction contraction: [cS, c] zero-padded 32->128
KCR = 2 * N_EXP                # 32 real correction rows

F32 = mybir.dt.float32
BF16 = mybir.dt.bfloat16
F8 = mybir.dt.float8e4
E4M3 = ml_dtypes.float8_e4m3
DR = mybir.MatmulPerfMode.DoubleRow


def _build_kernel():
    nc = bacc.Bacc(
        "TRN2",
        target_bir_lowering=False,
        debug=False,
        num_devices=N_CORES,
    )
    # corr = [lcorr/c0 (32 x 512) | rcorr (32 x 1024)] merged: one DMA.
    x8 = nc.declare_dram_parameter("x8", [P, KP, 2, BS], F8, isOutput=False)
    w8 = nc.declare_dram_parameter("w8", [N_EXP, P, KP, 2, OUT_DIM], F8, isOutput=False)
    c = nc.declare_dram_parameter("c", [P, BT, N_EXP], F32, isOutput=False)
    corr = nc.declare_dram_parameter("corr", [KC, BS + OUT_DIM], BF16, isOutput=False)
    out = nc.declare_dram_parameter("out", [BS, OUT_DIM], F32, isOutput=True)

    with ExitStack() as ctx:
        tc = ctx.enter_context(tile.TileContext(nc))
        const = ctx.enter_context(tc.tile_pool(name="const", bufs=1))
        accp = ctx.enter_context(tc.tile_pool(name="accp", bufs=1))
        wpool = ctx.enter_context(tc.tile_pool(name="wpool", bufs=3))
        psum = ctx.enter_context(tc.tile_pool(name="psum", bufs=4, space="PSUM"))

        # --- input DMAs (startup-critical first) -----------------------
        # The tiny correction tensors go first: they unblock the correction
        # matmuls early. w8[0] and x8 follow in halves, with the second
        # halves gated behind the first so the startup-critical first MiB
        # gets the full HBM bandwidth.
        # small tensors first (they unblock the correction matmuls inside
        # the junk-warmup window); then the first halves of w0+x8 with the
        # full gating chain so no later transfer steals their bandwidth.
        corr_sb = const.tile([KC, BS + OUT_DIM], BF16, tag="corr_sb")
        nc.sync.dma_start(corr_sb[:], corr[:, :])
        c_sb = const.tile([P, BT, N_EXP], F32, tag="c_sb")
        nc.sync.dma_start(c_sb[:], c[:, :])
        w0_sb = wpool.tile([P, KP, 2, OUT_DIM], F8, name="w_sb", tag="w_sb")
        x8_sb = const.tile([P, KP, 2, BS], F8, tag="x8_sb")
        dma_w0a = nc.sync.dma_start(w0_sb[:, 0:2], w8[0, :, 0:2])
        dma_x8a = nc.sync.dma_start(x8_sb[:, 0:2], x8[:, 0:2])
        dma_w0b = nc.sync.dma_start(w0_sb[:, 2:4], w8[0, :, 2:4])
        add_dep_helper(dma_w0b.ins, dma_x8a.ins, sync=True,
                       reason="full bandwidth for first halves of w0+x8")
        dma_x8b = nc.sync.dma_start(x8_sb[:, 2:4], x8[:, 2:4])
        add_dep_helper(dma_x8b.ins, dma_x8a.ins, sync=True,
                       reason="second halves ride together after the first")

        # --- PE clock pre-warm -----------------------------------------
        # ~28 tiny matmuls on a zeroed SBUF tile (no DMA dependency) keep
        # the PE continuously busy from engine boot, so the clock is at
        # full p-state before the first real matmul issues.
        warm_in = const.tile([KC, P], BF16, tag="warm_in")
        nc.vector.memset(warm_in[:], 0.0)
        warm_ps = psum.tile([P, NO, FD], F32, name="warm", tag="zp")
        for _ in range(28):
            nc.tensor.matmul(
                warm_ps[:, 0, 0:P],
                lhsT=warm_in[:],
                rhs=warm_in[:],
                start=True,
                stop=True,
            )

        lcorr_sb = corr_sb[:, 0:BS]
        rcorr_sb = corr_sb[:, BS : BS + OUT_DIM]

        acc = [
            accp.tile([P, NO, FD], F32, name=f"acc_{bt}", tag=f"acc_{bt}")
            for bt in range(BT)
        ]

        out_r = out[:, :].rearrange("(bt p) o -> p bt o", p=P)
        w_prev_dma = dma_w0b
        for n in range(N_EXP - 2):
            if n == 0:
                w_sb = w0_sb
            else:
                w_sb = wpool.tile([P, KP, 2, OUT_DIM], F8, name="w_sb", tag="w_sb")
                dma = nc.sync.dma_start(w_sb[:], w8[n, :, :])
                if n == 1:
                    # keep HBM focused on the startup-critical w8[0] + x8
                    add_dep_helper(
                        dma.ins, dma_x8b.ins, sync=True,
                        reason="gate w8[1] prefetch behind startup-critical w8[0]+x8",
                    )
            last = n == N_EXP - 1
            if n == 0:
                # Expert 0: correction matmuls (start=True) open all four
                # PSUM groups as soon as the small tensors land, then the
                # fp8 stream consumes the w8[0]/x8 halves as they arrive
                # (kp01 phase on the first halves, kp23 on the second).
                # acc = (corr/c0 + z_0) * c0  — c0 division was done on host.
                zps = [
                    psum.tile([P, NO, FD], F32, name="zp", tag="zp")
                    for _ in range(BT)
                ]
                for bt in range(BT):
                    for ot in range(NO):
                        nc.tensor.matmul(
                            zps[bt][:, ot],
                            lhsT=lcorr_sb[:, bt * P : (bt + 1) * P],
                            rhs=rcorr_sb[:, ot * FD : (ot + 1) * FD],
                            start=True,
                            stop=False,
                            skip_group_check=True,
                        )
                for kp_half in (range(0, 2), range(2, KP)):
                    for bt in range(BT):
                        for kp in kp_half:
                            for ot in range(NO):
                                nc.tensor.matmul(
                                    zps[bt][:, ot],
                                    lhsT=x8_sb[:, kp, :, bt * P : (bt + 1) * P],
                                    rhs=w_sb[:, kp, :, ot * FD : (ot + 1) * FD],
                                    start=False,
                                    stop=(kp == KP - 1),
                                    perf_mode=DR,
                                    skip_group_check=True,
                                )
                for bt in range(BT):
                    nc.vector.scalar_tensor_tensor(
                        out=acc[bt][:],
                        in0=zps[bt][:],
                        scalar=c_sb[:, bt, 0:1],
                        in1=acc[bt][:],
                        op0=mybir.AluOpType.mult,
                        op1=mybir.AluOpType.bypass,
                    )
            elif not last:
                for bt in range(BT):
                    zp = psum.tile([P, NO, FD], F32, name="zp", tag="zp")
                    for kp in range(KP):
                        for ot in range(NO):
                            nc.tensor.matmul(
                                zp[:, ot],
                                lhsT=x8_sb[:, kp, :, bt * P : (bt + 1) * P],
                                rhs=w_sb[:, kp, :, ot * FD : (ot + 1) * FD],
                                start=(kp == 0),
                                stop=(kp == KP - 1),
                                perf_mode=DR,
                            )
                    # acc += z * c[:, n]  (fused on DVE; c per-partition scalar)
                    nc.vector.scalar_tensor_tensor(
                        out=acc[bt][:],
                        in0=zp[:],
                        scalar=c_sb[:, bt, n : n + 1],
                        in1=acc[bt][:],
                        op0=mybir.AluOpType.mult,
                        op1=mybir.AluOpType.add,
                    )
        # --- experts 14+15 merged per-bt ------------------------------
        # Interleaving the last two experts lets each bt's final combine /
        # relu / store chain drain while the other expert's matmuls run,
        # instead of stacking the whole tail after the last matmul.
        n14, n15 = N_EXP - 2, N_EXP - 1
        w14_sb = wpool.tile([P, KP, 2, OUT_DIM], F8, name="w_sb", tag="w_sb")
        nc.sync.dma_start(w14_sb[:], w8[n14, :, :])
        w15_sb = wpool.tile([P, KP, 2, OUT_DIM], F8, name="w_sb", tag="w_sb")
        nc.sync.dma_start(w15_sb[:], w8[n15, :, :])
        for bt in range(BT):
            last_bt = bt == BT - 1
            zp_a = psum.tile([P, NO, FD], F32, name="zp", tag="zp")
            for kp in range(KP):
                for ot in range(NO):
                    nc.tensor.matmul(
                        zp_a[:, ot],
                        lhsT=x8_sb[:, kp, :, bt * P : (bt + 1) * P],
                        rhs=w14_sb[:, kp, :, ot * FD : (ot + 1) * FD],
                        start=(kp == 0),
                        stop=(kp == KP - 1),
                        perf_mode=DR,
                    )
            nc.vector.scalar_tensor_tensor(
                out=acc[bt][:],
                in0=zp_a[:],
                scalar=c_sb[:, bt, n14 : n14 + 1],
                in1=acc[bt][:],
                op0=mybir.AluOpType.mult,
                op1=mybir.AluOpType.add,
            )
            zp_b = psum.tile([P, NO, FD], F32, name="zp", tag="zp")
            for ot in range(NO):
                for kp in range(KP):
                    nc.tensor.matmul(
                        zp_b[:, ot],
                        lhsT=x8_sb[:, kp, :, bt * P : (bt + 1) * P],
                        rhs=w15_sb[:, kp, :, ot * FD : (ot + 1) * FD],
                        start=(kp == 0),
                        stop=(kp == KP - 1),
                        perf_mode=DR,
                    )
                nc.vector.scalar_tensor_tensor(
                    out=acc[bt][:, ot],
                    in0=zp_b[:, ot],
                    scalar=c_sb[:, bt, n15 : n15 + 1],
                    in1=acc[bt][:, ot],
                    op0=mybir.AluOpType.mult,
                    op1=mybir.AluOpType.add,
                )
                if last_bt:
                    nc.scalar.activation(
                        acc[bt][:, ot],
                        acc[bt][:, ot],
                        mybir.ActivationFunctionType.Relu,
                    )
                    nc.sync.dma_start(
                        out_r[:, bt, ot * FD : (ot + 1) * FD],
                        acc[bt][:, ot],
                    )
            if not last_bt:
                nc.scalar.activation(
                    acc[bt][:],
                    acc[bt][:],
                    mybir.ActivationFunctionType.Relu,
                )
                nc.sync.dma_start(out_r[:, bt], acc[bt][:])

    nc.compile()
    return nc


def _prep_weights(weight, bias):
    """Host-side format prep (shared across cores): center + quantize W."""
    w = np.asarray(weight, dtype=np.float32)
    bias = np.ascontiguousarray(np.asarray(bias, dtype=np.float32))
    m = w.mean(axis=1, dtype=np.float64).astype(np.float32)     # [n, o]
    v8 = (w - m[:, None, :]).astype(E4M3)                       # [n, i, o]
    # device layout [n, p, kp, two, o] with i = kp*256 + two*128 + p
    w_dev = np.ascontiguousarray(
        v8.reshape(N_EXP, KP, 2, P, OUT_DIM).transpose(0, 3, 1, 2, 4)
    )
    rcorr = np.concatenate(
        [m, bias, np.zeros((KC - KCR, OUT_DIM), np.float32)], axis=0
    )                                                           # [128, o]
    return w_dev, rcorr


def _prep_shard(xs, cs, rcorr):
    """Host-side format prep for one core's batch shard."""
    xs = np.asarray(xs, dtype=np.float32)
    cs = np.ascontiguousarray(np.asarray(cs, dtype=np.float32))
    # lhsT layout [p, kp, two, b] with i = kp*256 + two*128 + p
    x_dev = np.ascontiguousarray(
        xs.T.reshape(KP, 2, P, BS).transpose(2, 0, 1, 3).astype(E4M3)
    )
    S = xs.sum(axis=1, dtype=np.float64).astype(np.float32)     # [b]
    lcorr = np.concatenate(
        [cs * S[:, None], cs, np.zeros((BS, KC - KCR), np.float32)], axis=1
    ).T                                                         # [128, b]
    lcorr_e0 = lcorr / cs[:, 0]                                 # fold: /c0
    corr_dev = np.ascontiguousarray(
        np.concatenate([lcorr_e0, rcorr], axis=1).astype(ml_dtypes.bfloat16)
    )                                                           # [32, b+o]
    c_dev = np.ascontiguousarray(cs.reshape(BT, P, N_EXP).transpose(1, 0, 2))
    return x_dev, corr_dev, c_dev


_NC_CACHE = {}


def _get_nc():
    if "nc" not in _NC_CACHE:
        _NC_CACHE["nc"] = _build_kernel()
    return _NC_CACHE["nc"]


def _run(x, comp_weight, weight, bias, trace=False):
    x = np.ascontiguousarray(np.asarray(x, dtype=np.float32))
    comp_weight = np.ascontiguousarray(np.asarray(comp_weight, dtype=np.float32))
    w_dev, rcorr = _prep_weights(weight, bias)

    in_maps = []
    for r in range(N_CORES):
        sl = slice(r * BS, (r + 1) * BS)
        x_dev, corr_dev, c_dev = _prep_shard(x[sl], comp_weight[sl], rcorr)
        in_maps.append(
            {
                "x8": x_dev,
                "w8": w_dev,
                "c": c_dev,
                "corr": corr_dev,
            }
        )
    res = run_bass_kernel_spmd(
        _get_nc(), in_maps, core_ids=list(range(N_CORES)), trace=trace
    )
    out = np.concatenate([res.results[r]["out"] for r in range(N_CORES)], axis=0)
    return out, res


def kernel(x, comp_weight, weight, bias):
    out, _ = _run(x, comp_weight, weight, bias)
    return out
